# revision 23
# baseline (speedup 1.0000x reference)
"""Trainium2 Bass kernel for nn_CrossAttentionForQA (self-contained).

One transformer cross-attention QA layer: QKV proj -> masked MHA -> out proj
-> add&LN -> FFN(gelu) -> add&LN, for B=8, S=1024, E=1024, H=16, F=4096.

Sharding: data-parallel over batch, one batch element per NeuronCore (8 cores,
no collectives). On-device activations live feature-on-partitions (transposed,
[E, S]); inputs are pre-transposed on the host and the output is transposed
back on the host.

Numerics: bf16 GEMM operands with fp32 PSUM accumulation; softmax without
max-subtraction (scores are provably small for this operator); the pairwise
additive mask am[q]&am[k] is folded into the score GEMM as one extra
contraction row carrying am; the key mask is an exp bias of -1e4 per masked
key row; softmax denominators come from an extra all-ones column in the V
stationary operand; LayerNorm stats via ones-matmul on the tensor engine.
Residual/LN carriers (y, h1, y2) bounce through DRAM scratch to keep SBUF
pool lifetimes strictly LIFO.
"""

from contextlib import ExitStack

import numpy as np
import ml_dtypes

import concourse.bass as bass
import concourse.tile as tile
from concourse import bacc, mybir
from concourse.bass_utils import run_bass_kernel_spmd

B, S, E, H, F = 8, 1024, 1024, 16, 4096
HD = E // H          # 64
P = 128
ET = E // P          # 8  E-tiles
FT = F // P          # 32 F-tiles
NH = 512             # matmul free-dim chunk (one PSUM bank of fp32)
EPS = 1e-12
QNEG = -60.0         # exp(score + QNEG) ~ 1e-25: negligible vs denom >= 255,
                     # and score+QNEG stays inside the ScalarE exp LUT range

bf = mybir.dt.bfloat16
f32 = mybir.dt.float32
AF = mybir.ActivationFunctionType
OP = mybir.AluOpType
bf16np = ml_dtypes.bfloat16

_CACHE: dict = {}


def _build(nc: bass.Bass):
    # ---------------- DRAM parameters (per core) ----------------
    xT_d = nc.declare_dram_parameter("xT", [E, S], f32, False)       # x transposed
    xTb_d = nc.declare_dram_parameter("xTb", [E, S], bf, False)      # x^T in bf16
    w1_d = nc.declare_dram_parameter("w1", [E, 3 * E], bf, False)    # q-part /8
    wo_d = nc.declare_dram_parameter("wo", [E, E], bf, False)
    win_d = nc.declare_dram_parameter("win", [E, F], bf, False)
    wout_d = nc.declare_dram_parameter("wout", [F, E], bf, False)
    amb_d = nc.declare_dram_parameter("amb", [S], bf, False)         # attn mask 0/1
    amc_d = nc.declare_dram_parameter("amc", [S], bf, False)         # am / 32
    bvb_d = nc.declare_dram_parameter("bvb", [P, E], f32, False)     # v-bias bcast
    ppq_d = nc.declare_dram_parameter("ppq", [P, ET], f32, False)    # b1 q-part /8
    ppk_d = nc.declare_dram_parameter("ppk", [P, ET], f32, False)    # b1 k-part
    ppo_d = nc.declare_dram_parameter("ppo", [P, ET], f32, False)    # out_proj_b
    ppi_d = nc.declare_dram_parameter("ppi", [P, FT], f32, False)    # b_in
    ppu_d = nc.declare_dram_parameter("ppu", [P, ET], f32, False)    # b_out
    ppw_d = nc.declare_dram_parameter("ppw", [P, ET], f32, False)    # ln_w
    ppb_d = nc.declare_dram_parameter("ppb", [P, ET], f32, False)    # ln_b
    ppm_d = nc.declare_dram_parameter("ppm", [P, ET], f32, False)    # key-mask bias
    out_d = nc.declare_dram_parameter("outT", [E, S], f32, True)

    # DRAM scratch for residual carriers
    yf_d = nc.dram_tensor("yf_s", [E, S], f32)
    h1_d = nc.dram_tensor("h1_s", [E, S], f32)
    y2_d = nc.dram_tensor("y2_s", [E, S], f32)

    def r3(d):  # [E,S] dram -> [P, ET, S] tiled view
        return d.rearrange("(t p) s -> p t s", p=P)

    # small DRAM scratch rows used to broadcast a [1, S] vector across
    # partitions (DMA out, then DMA back with a partition-broadcast view;
    # SBUF APs cannot partition-broadcast but DRAM APs can)
    bscr = [nc.dram_tensor(f"bscr{i}", [S], f32) for i in range(4)]
    _bn = [0]

    def bcast(src_row, dst_ap, rows):
        scr = bscr[_bn[0] % len(bscr)]
        _bn[0] += 1
        nc.sync.dma_start(scr[None, :], src_row)
        nc.sync.dma_start(dst_ap, scr[None, :].broadcast_to([rows, S]))

    with tile.TileContext(nc) as tc:
        with ExitStack() as root:
            const = root.enter_context(tc.tile_pool(name="const", bufs=1))
            mmp = root.enter_context(tc.tile_pool(name="mmp", bufs=2, space="PSUM"))
            ctxp = root.enter_context(tc.tile_pool(name="ctxp", bufs=2, space="PSUM"))

            # ------------- constants -------------
            ppq = const.tile([P, ET], f32, tag="ppq")
            ppk = const.tile([P, ET], f32, tag="ppk")
            ppo = const.tile([P, ET], f32, tag="ppo")
            ppi = const.tile([P, FT], f32, tag="ppi")
            ppu = const.tile([P, ET], f32, tag="ppu")
            ppw = const.tile([P, ET], f32, tag="ppw")
            ppb = const.tile([P, ET], f32, tag="ppb")
            ppm = const.tile([P, ET], f32, tag="ppm")
            bvbs = const.tile([P, E], f32, tag="bvbs")
            onesml = const.tile([P, 2], bf, tag="ones")  # col0: 1/1024
            epst = const.tile([1, 1], f32, tag="eps")
            for i, (tt, dd) in enumerate(
                    ((ppq, ppq_d), (ppk, ppk_d), (ppo, ppo_d), (ppi, ppi_d),
                     (ppu, ppu_d), (ppw, ppw_d), (ppb, ppb_d), (ppm, ppm_d),
                     (bvbs, bvb_d))):
                nc.sync.dma_start(tt[:], dd[:])
                if i % 3 == 2:
                    # mini-barrier chain: keeps the per-instruction semaphore
                    # wait count under the ISA limit (waits consolidate
                    # transitively through each barrier)
                    tc.strict_bb_all_engine_barrier()
            nc.vector.memset(onesml[:, 0:1], 1.0 / 1024.0)
            nc.vector.memset(onesml[:, 1:2], 1.0)
            nc.vector.memset(epst[:], float(EPS))
            tc.strict_bb_all_engine_barrier()

            with tc.tile_pool(name="pctx", bufs=1) as pctx:
                ctxT = pctx.tile([P, ET, S], bf, tag="ctxT")
                with tc.tile_pool(name="pqkv", bufs=1) as pqkv:
                    qhat = pqkv.tile([P, H, S], bf, tag="qhat")
                    khat = pqkv.tile([P, H, S], bf, tag="khat")
                    vhat = pqkv.tile([P, ET, H, HD + 1], bf, tag="vhat")

    # mask rows / zero padding; ones column in vhat.
                    # Head parity layout inside each [128, S] block (all
                    # partition bases 32-aligned). The pairwise mask term
                    # am[q]&am[k] enters the score contraction through a
                    # 32-row band holding am/32 (qhat) x am (khat):
                    # sum over the band = 32 * (am/32) * am = am*am, exact
                    # in bf16 because 1/32 is a power of two.
                    #   even head: data rows 0:64, band 64:96, zeros 96:128
                    #   odd head:  zeros 0:32, band 32:64, data rows 64:128
                    for t, band in ((qhat, amc_d), (khat, amb_d)):
                        ev = t.rearrange("p (hp two) s -> p hp two s", two=2)
                        nc.vector.memset(ev[96:P, :, 0, :], 0.0)
                        nc.vector.memset(ev[0:32, :, 1, :], 0.0)
                        nc.sync.dma_start(
                            ev[64:96, :, 0, :],
                            band[None, None, :].broadcast_to([32, H // 2, S]),
                        )
                        nc.sync.dma_start(
                            ev[32:64, :, 1, :],
                            band[None, None, :].broadcast_to([32, H // 2, S]),
                        )
                        tc.strict_bb_all_engine_barrier()
                    nc.vector.memset(vhat[:, :, :, HD:HD + 1], 1.0)

                    # consolidate init deps (memsets, mask bands, consts) so
                    # later instructions don't exceed per-inst sync-wait slots
                    tc.strict_bb_all_engine_barrier()

                    # ---- phase 1: QKV projections ----
                    with tc.tile_pool(name="pw1", bufs=1) as pw1:
                        xbf = pw1.tile([P, ET, S], bf, tag="xbf")
                        w1s = pw1.tile([P, ET, 3 * E], bf, tag="w1s")
                        nc.sync.dma_start(xbf[:], r3(xTb_d))
                        nc.sync.dma_start(
                            w1s[:], w1_d.rearrange("(t p) f -> p t f", p=P)
                        )

                        # q^T, k^T: [feat_tile, sq] = W.T @ x
                        for tf in range(2 * ET):
                            isq = tf < ET
                            t = tf % ET
                            foff = t * P if isq else E + t * P
                            ps = mmp.tile([P, S], f32, tag="mm")
                            for half in range(2):
                                for kt in range(ET):
                                    nc.tensor.matmul(
                                        ps[:, half * NH:(half + 1) * NH],
                                        lhsT=w1s[:, kt, foff:foff + P],
                                        rhs=xbf[:, kt, half * NH:(half + 1) * NH],
                                        start=(kt == 0),
                                        stop=(kt == ET - 1),
                                    )
                            dst = qhat if isq else khat
                            pp = ppq if isq else ppk
                            nc.vector.tensor_scalar_add(
                                dst[0:HD, 2 * t, :], ps[0:HD, :], pp[0:HD, t:t + 1]
                            )
                            nc.vector.tensor_scalar_add(
                                dst[HD:P, 2 * t + 1, :], ps[HD:P, :], pp[HD:P, t:t + 1]
                            )

                        # v natural: [sq_tile, feat] = x @ Wv
                        for st in range(ET):
                            ps = mmp.tile([P, E], f32, tag="mm")
                            for half in range(2):
                                for kt in range(ET):
                                    nc.tensor.matmul(
                                        ps[:, half * NH:(half + 1) * NH],
                                        lhsT=xbf[:, kt, st * P:(st + 1) * P],
                                        rhs=w1s[:, kt,
                                                2 * E + half * NH:
                                                2 * E + (half + 1) * NH],
                                        start=(kt == 0),
                                        stop=(kt == ET - 1),
                                    )
                            nc.vector.tensor_tensor(
                                vhat[:, st, :, 0:HD],
                                ps.rearrange("p (h d) -> p h d", d=HD),
                                bvbs.rearrange("p (h d) -> p h d", d=HD),
                                OP.add,
                            )

                    # ---- phase 2: attention ----
                    with tc.tile_pool(name="patt", bufs=2) as attw:
                        for h in range(H):
                            cx = ctxp.tile([P, S], f32, tag="ctx")
                            for skt in range(ET):
                                sc = mmp.tile([P, S], f32, tag="mm")
                                for half in range(2):
                                    nc.tensor.matmul(
                                        sc[:, half * NH:(half + 1) * NH],
                                        lhsT=khat[:, h, skt * P:(skt + 1) * P],
                                        rhs=qhat[:, h, half * NH:(half + 1) * NH],
                                        start=True,
                                        stop=True,
                                    )
                                pb = attw.tile([P, S], bf, tag="probs", bufs=3)
                                nc.scalar.activation(
                                    pb[:], sc[:], AF.Exp, bias=ppm[:, skt:skt + 1]
                                )
                                for half in range(2):
                                    nc.tensor.matmul(
                                        cx[0:HD + 1, half * NH:(half + 1) * NH],
                                        lhsT=vhat[:, skt, h, :],
                                        rhs=pb[:, half * NH:(half + 1) * NH],
                                        start=(skt == 0),
                                        stop=(skt == ET - 1),
                                    )
                            # rows 0:64 = ctx_u, row 64 = softmax denominator
                            rc = attw.tile([P, S], f32, tag="rc")
                            nc.vector.reciprocal(rc[HD:HD + 1, :], cx[HD:HD + 1, :])
                            rb = attw.tile([P, S], f32, tag="rb")
                            bcast(rc[HD:HD + 1, :], rb[0:HD, :], HD)
                            if h % 2 == 0:
                                nc.vector.tensor_tensor(
                                    ctxT[0:HD, h // 2, :], cx[0:HD, :], rb[0:HD, :],
                                    OP.mult,
                                )
                            else:
                                tmp = attw.tile([HD, S], bf, tag="octx")
                                nc.vector.tensor_tensor(
                                    tmp[:], cx[0:HD, :], rb[0:HD, :], OP.mult
                                )
                                nc.sync.dma_start(ctxT[HD:P, h // 2, :], tmp[:])

                # ---- phase 3: out proj (-> y to DRAM) ----
                with tc.tile_pool(name="pout", bufs=2) as pout:
                    for ft in range(ET):
                        wt = pout.tile([P, ET, P], bf, tag="wo", bufs=3)
                        nc.sync.dma_start(
                            wt[:],
                            wo_d.rearrange("(t p) f -> p t f", p=P)[
                                :, :, ft * P:(ft + 1) * P
                            ],
                        )
                        ps = mmp.tile([P, S], f32, tag="mm")
                        for half in range(2):
                            for kt in range(ET):
                                nc.tensor.matmul(
                                    ps[:, half * NH:(half + 1) * NH],
                                    lhsT=wt[:, kt, :],
                                    rhs=ctxT[:, kt, half * NH:(half + 1) * NH],
                                    start=(kt == 0),
                                    stop=(kt == ET - 1),
                                )
                        tv = pout.tile([P, S], f32, tag="tv")
                        nc.scalar.activation(
                            tv[:], ps[:], AF.Identity, bias=ppo[:, ft:ft + 1]
                        )
                        xt = pout.tile([P, S], f32, tag="xt")
                        nc.sync.dma_start(xt[:], r3(xT_d)[:, ft, :])
                        yt = pout.tile([P, S], f32, tag="yt")
                        nc.vector.tensor_tensor(yt[:], tv[:], xt[:], OP.add)
                        nc.sync.dma_start(r3(yf_d)[:, ft, :], yt[:])

            # ---- phase 3b/5: LN1; phase 4: FFN; LN2 -> out ----
            with tc.tile_pool(name="pg", bufs=1) as pg:
                gT = pg.tile([P, FT, S], bf, tag="gT")
                with tc.tile_pool(name="ph1b", bufs=1) as ph1b:
                    h1bf = ph1b.tile([P, ET, S], bf, tag="h1bf")

                    _layernorm(nc, tc, ctxp, yf_d, h1_d, h1bf, bcast,
                               onesml, epst, ppw, ppb, r3)

                    # FFN GEMM1 + gelu
                    with tc.tile_pool(name="pg1", bufs=3) as pg1:
                        for ftile in range(FT):
                            wt = pg1.tile([P, ET, P], bf, tag="win")
                            nc.sync.dma_start(
                                wt[:],
                                win_d.rearrange("(t p) f -> p t f", p=P)[
                                    :, :, ftile * P:(ftile + 1) * P
                                ],
                            )
                            ps = mmp.tile([P, S], f32, tag="mm")
                            for half in range(2):
                                for kt in range(ET):
                                    nc.tensor.matmul(
                                        ps[:, half * NH:(half + 1) * NH],
                                        lhsT=wt[:, kt, :],
                                        rhs=h1bf[:, kt, half * NH:(half + 1) * NH],
                                        start=(kt == 0),
                                        stop=(kt == ET - 1),
                                    )
                            nc.scalar.activation(
                                gT[:, ftile, :], ps[:], AF.Gelu,
                                bias=ppi[:, ftile:ftile + 1],
                            )

                # FFN GEMM2 (-> y2 to DRAM)
                with tc.tile_pool(name="pg2", bufs=2) as pg2:
                    for et in range(ET):
                        wt2 = pg2.tile([P, FT, P], bf, tag="wout", bufs=3)
                        nc.sync.dma_start(
                            wt2[:],
                            wout_d.rearrange("(t p) f -> p t f", p=P)[
                                :, :, et * P:(et + 1) * P
                            ],
                        )
                        ps = mmp.tile([P, S], f32, tag="mm")
                        for half in range(2):
                            for kt in range(FT):
                                nc.tensor.matmul(
                                    ps[:, half * NH:(half + 1) * NH],
                                    lhsT=wt2[:, kt, :],
                                    rhs=gT[:, kt, half * NH:(half + 1) * NH],
                                    start=(kt == 0),
                                    stop=(kt == FT - 1),
                                )
                        tv = pg2.tile([P, S], f32, tag="tv")
                        nc.scalar.activation(
                            tv[:], ps[:], AF.Identity, bias=ppu[:, et:et + 1]
                        )
                        ht = pg2.tile([P, S], f32, tag="ht")
                        nc.sync.dma_start(ht[:], r3(h1_d)[:, et, :])
                        yt = pg2.tile([P, S], f32, tag="yt")
                        nc.vector.tensor_tensor(yt[:], tv[:], ht[:], OP.add)
                        nc.sync.dma_start(r3(y2_d)[:, et, :], yt[:])

            _layernorm(nc, tc, ctxp, y2_d, out_d, None, bcast,
                       onesml, epst, ppw, ppb, r3)

    return nc


def _layernorm(nc, tc, ctxp, src_d, dst_d, hbf, bcast, onesml, epst,
               ppw, ppb, r3):
    """LN over the feature axis (partitions+tiles) of src_d [E, S] fp32 DRAM.

    Stats: mu and E[y^2] via scaled-ones matmuls over streamed bf16 casts;
    var = E[y^2] - mu^2. Normalized output is DMAd to dst_d (fp32); if hbf is
    given, a bf16 copy is also written there (SBUF, for the next GEMM).
    """
    with tc.tile_pool(name="pln", bufs=2) as pln:
        mups = ctxp.tile([1, S], f32, tag="ctx")
        eyps = ctxp.tile([1, S], f32, tag="ctx")
        for kt in range(ET):
            yt = pln.tile([P, S], f32, tag="ys", bufs=3)
            nc.sync.dma_start(yt[:], r3(src_d)[:, kt, :])
            yb = pln.tile([P, S], bf, tag="yb", bufs=3)
            nc.vector.tensor_copy(out=yb[:], in_=yt[:])
            for half in range(2):
                nc.tensor.matmul(
                    mups[:, half * NH:(half + 1) * NH],
                    lhsT=onesml[:, 0:1],
                    rhs=yb[:, half * NH:(half + 1) * NH],
                    start=(kt == 0),
                    stop=(kt == ET - 1),
                )
            nc.scalar.activation(yb[:], yb[:], AF.Square)
            for half in range(2):
                nc.tensor.matmul(
                    eyps[:, half * NH:(half + 1) * NH],
                    lhsT=onesml[:, 0:1],
                    rhs=yb[:, half * NH:(half + 1) * NH],
                    start=(kt == 0),
                    stop=(kt == ET - 1),
                )
        mu = pln.tile([1, S], f32, tag="mu")
        rr = pln.tile([1, S], f32, tag="rr")
        nc.vector.tensor_copy(out=mu[:], in_=mups[:])
        nc.vector.tensor_tensor(rr[:], mu[:], mu[:], OP.mult)
        nc.vector.tensor_tensor(rr[:], eyps[:], rr[:], OP.subtract)
        nc.scalar.activation(rr[:], rr[:], AF.Sqrt, bias=epst[:])
        nc.vector.reciprocal(rr[:], rr[:])
        mub = pln.tile([P, S], f32, tag="mub", bufs=1)
        rb2 = pln.tile([P, S], f32, tag="rb2", bufs=1)
        bcast(mu[:], mub[:], P)
        bcast(rr[:], rb2[:], P)
        for t in range(ET):
            yt = pln.tile([P, S], f32, tag="ys", bufs=3)
            nc.sync.dma_start(yt[:], r3(src_d)[:, t, :])
            tv = pln.tile([P, S], f32, tag="lt")
            nc.vector.tensor_tensor(tv[:], yt[:], mub[:], OP.subtract)
            nc.vector.tensor_tensor(tv[:], tv[:], rb2[:], OP.mult)
            ov = pln.tile([P, S], f32, tag="ov")
            nc.vector.tensor_scalar(
                ov[:], tv[:], ppw[:, t:t + 1], ppb[:, t:t + 1], OP.mult, OP.add
            )
            nc.sync.dma_start(r3(dst_d)[:, t, :], ov[:])
            if hbf is not None:
                nc.vector.tensor_copy(out=hbf[:, t, :], in_=ov[:])


def get_nc():
    if "nc" not in _CACHE:
        # Bacc (not plain Bass): its compile() pass splits semaphore waits to
        # the TRN2 limit of one wait per instruction (generate_event_semaphores)
        nc = bacc.Bacc("TRN2")
        _build(nc)
        nc.finalize()
        _CACHE["nc"] = nc
    return _CACHE["nc"]


def _strided_pp(v: np.ndarray) -> np.ndarray:
    """[n*128] feature vector -> [128, n] per-partition layout (col t = tile t)."""
    return np.ascontiguousarray(v.reshape(-1, P).T.astype(np.float32))


def make_in_maps(inputs: dict) -> list[dict]:
    x = np.asarray(inputs["final_hidden_state"], np.float32)
    am_i = np.asarray(inputs["attention_mask"]) != 0
    tt = np.asarray(inputs["token_type_ids"])

    w1 = np.array(np.asarray(inputs["in_proj_w"], np.float32))
    b1 = np.array(np.asarray(inputs["in_proj_b"], np.float32))
    w1[:, 0:E] /= 8.0
    b1q = b1[0:E] / 8.0

    shared = {
        "w1": w1.astype(bf16np),
        "wo": np.asarray(inputs["out_proj_w"], np.float32).astype(bf16np),
        "win": np.asarray(inputs["w_in"], np.float32).astype(bf16np),
        "wout": np.asarray(inputs["w_out"], np.float32).astype(bf16np),
        "ppq": _strided_pp(b1q),
        "ppk": _strided_pp(b1[E:2 * E]),
        "ppo": _strided_pp(np.asarray(inputs["out_proj_b"], np.float32)),
        "ppi": _strided_pp(np.asarray(inputs["b_in"], np.float32)),
        "ppu": _strided_pp(np.asarray(inputs["b_out"], np.float32)),
        "ppw": _strided_pp(np.asarray(inputs["ln_w"], np.float32)),
        "ppb": _strided_pp(np.asarray(inputs["ln_b"], np.float32)),
        "bvb": np.ascontiguousarray(
            np.broadcast_to(b1[2 * E:3 * E][None, :], (P, E)).astype(np.float32)
        ),
    }
    qm = (tt == 1) | (~am_i)
    qm[:, 0] = True
    maps = []
    for b in range(B):
        m = dict(shared)
        xT = np.ascontiguousarray(x[b].T)
        m["xT"] = xT
        m["xTb"] = xT.astype(bf16np)
        m["amb"] = am_i[b].astype(bf16np)
        m["amc"] = (am_i[b].astype(np.float32) / 32.0).astype(bf16np)
        m["ppm"] = _strided_pp(np.where(qm[b], np.float32(QNEG), np.float32(0.0)))
        maps.append(m)
    return maps


def run(inputs: dict, trace: bool = False):
    nc = get_nc()
    res = run_bass_kernel_spmd(nc, make_in_maps(inputs), list(range(B)), trace=trace)
    out = np.stack([np.asarray(r["outT"], np.float32).T for r in res.results])
    return out, res


def kernel(**inputs) -> np.ndarray:
    out, _ = run(inputs)
    return out
